# revision 21
# baseline (speedup 1.0000x reference)
"""Trainium2 Bass kernel for CustomBCELoss.

Reference semantics (per torch BCELoss with per-channel weighting):
    p, t flattened channel-first to (C=3, M=8388608)
    ones[c]   = count_nonzero(t[c])
    weight[c] = M / max(ones[c], 1)  if ones[c] > 0 else 1000.0
    bce[c]    = -mean(t*max(log p, -100) + (1-t)*max(log1p(-p), -100))
    out       = mean(weight * bce)

Since t ∈ {0,1}, the per-element term is log|p + t - 1|, and with
p ∈ [1e-4, 1-1e-4] (post-sigmoid probabilities) the -100 clamp never
fires: |p + t - 1| >= ~6e-5 so log >= ~-10.

8-way data-parallel over the flat element range. Per-core pipeline over
[128, 2048] tiles (tapered at both ends) with deep (bufs=6)
double-buffering so the ~430 GB/s/core HBM stream (25.2 MB) never
stalls on slot recycling. Engine balance, each stream < the 60 us DMA:
    Sync DGE : all HBM loads
    PE   : per-segment positive counts. t is exactly 0.0f/1.0f, so the
           strided bf16 view of its high 2 bytes is exactly 0.0/1.0 —
           single-pass bf16 matmuls (ones.T @ t) into PSUM, exact.
    DVE  : d = (p - 1) + t (fused); for SQUARE_TILES also u = d*d
           (their Ln accumulates 2*log|d|, halved on the host)
    ACT  : u = |d| for the remaining tiles; Ln(u) in place with fused
           per-partition accum_out. A dummy Ln in the preamble pins the
           natural_log table set (contains Abs+Ln): one load, preamble.
Tiles never cross an (n, c) half-block boundary, so per-tile/per-segment
partials map 1:1 to channels on the host, which applies the tiny
weight/mean epilogue in float64.
"""

import numpy as np

import concourse.bacc as bacc
import concourse.bass as bass
import concourse.tile as tile
from concourse import mybir
from concourse.bass_utils import run_bass_kernel_spmd

N_CORES = 8
C = 3
SPATIAL = 128 * 128 * 128            # elements per (n, c) block
N_BATCH = 4
FULL = N_BATCH * C * SPATIAL         # 25_165_824 total elements
PER_CORE = FULL // N_CORES           # 3_145_728
P = 128
# Per-partition column counts per tile; sum must equal PER_CORE / P = 24576.
TILE_F = [4096, 4096, 4096, 2048, 2048, 2048, 2048, 2048, 1024, 512, 512]
NTILES = len(TILE_F)
TILE_ELEMS = [P * f for f in TILE_F]
assert sum(TILE_ELEMS) == PER_CORE
# |d| as d*d on DVE for these tiles (rest: Abs on ACT), placed so that
# neither engine carries a backlog into the drain after the last DMA.
SQUARE_TILES = {1, 3, 5, 6, 8}
HALF_BLOCK_COLS = (SPATIAL // 2) // P          # 8192 cols per half-block
N_SEG = (PER_CORE // P) // HALF_BLOCK_COLS     # 3 segments per core
MM_N = 512                                      # matmul moving free dim
M_PER_CH = FULL // C                 # 8_388_608
EMPTY_WEIGHT = 1000.0

_NC_CACHE = None


def _build_nc():
    nc = bacc.Bacc(
        "TRN2", target_bir_lowering=False, debug=False, num_devices=N_CORES
    )
    p_in = nc.declare_dram_parameter(
        "p_in", [PER_CORE], mybir.dt.float32, isOutput=False
    )
    t_in = nc.declare_dram_parameter(
        "t_in", [PER_CORE], mybir.dt.float32, isOutput=False
    )
    vsum_out = nc.declare_dram_parameter(
        "vsum", [P, NTILES], mybir.dt.float32, isOutput=True
    )
    tsum_out = nc.declare_dram_parameter(
        "tsum", [1, N_SEG * MM_N], mybir.dt.float32, isOutput=True
    )

    seg_of_tile = []
    off = 0
    for f in TILE_F:
        assert off // HALF_BLOCK_COLS == (off + f - 1) // HALF_BLOCK_COLS
        seg_of_tile.append(off // HALF_BLOCK_COLS)
        off += f
    mm_total = {s: 0 for s in range(N_SEG)}
    for i, f in enumerate(TILE_F):
        mm_total[seg_of_tile[i]] += f // MM_N

    with tile.TileContext(nc) as tc:
        with (
            tc.tile_pool(name="pp", bufs=6) as p_pool,
            tc.tile_pool(name="tp", bufs=6) as t_pool,
            tc.tile_pool(name="res", bufs=1) as res_pool,
            tc.tile_pool(name="ps", bufs=1, space="PSUM") as ps_pool,
        ):
            ones_t = res_pool.tile([P, 1], mybir.dt.bfloat16)
            nc.vector.memset(ones_t, 1.0)
            vsum_t = res_pool.tile([P, NTILES], mybir.dt.float32)
            cnt_sb = res_pool.tile([1, N_SEG * MM_N], mybir.dt.float32)
            # Dummy Ln pins the natural_log table set (contains Abs too).
            warm_t = res_pool.tile([P, 1], mybir.dt.float32)
            nc.vector.memset(warm_t, 1.0)
            nc.scalar.activation(
                out=warm_t, in_=warm_t, func=mybir.ActivationFunctionType.Ln
            )
            psum_seg = [
                ps_pool.tile(
                    [1, MM_N], mybir.dt.float32, tag=f"seg{s}", name=f"psum_seg{s}"
                )
                for s in range(N_SEG)
            ]
            mm_done = {s: 0 for s in range(N_SEG)}
            off = 0
            for i, f in enumerate(TILE_F):
                n = P * f
                p_src = p_in[off : off + n].rearrange("(p f) -> p f", p=P)
                t_src = t_in[off : off + n].rearrange("(p f) -> p f", p=P)
                off += n
                s = seg_of_tile[i]
                p_t = p_pool.tile([P, f], mybir.dt.float32, tag="p")
                t_t = t_pool.tile([P, f], mybir.dt.float32, tag="t")
                nc.sync.dma_start(out=p_t, in_=p_src)
                nc.sync.dma_start(out=t_t, in_=t_src)
                t_hi = t_t[:].bitcast(mybir.dt.bfloat16).rearrange(
                    "p (f two) -> p f two", two=2
                )[:, :, 1]
                for j in range(f // MM_N):
                    nc.tensor.matmul(
                        out=psum_seg[s][:, :],
                        lhsT=ones_t[:, :],
                        rhs=t_hi[:, j * MM_N : (j + 1) * MM_N],
                        start=(mm_done[s] == 0),
                        stop=(mm_done[s] == mm_total[s] - 1),
                    )
                    mm_done[s] += 1
                # d = (p - 1) + t, in place into p_t
                nc.vector.scalar_tensor_tensor(
                    out=p_t,
                    in0=p_t,
                    scalar=1.0,
                    in1=t_t,
                    op0=mybir.AluOpType.subtract,
                    op1=mybir.AluOpType.add,
                )
                if i in SQUARE_TILES:
                    nc.vector.tensor_tensor(
                        out=p_t, in0=p_t, in1=p_t, op=mybir.AluOpType.mult
                    )
                else:
                    nc.scalar.activation(
                        out=p_t, in_=p_t, func=mybir.ActivationFunctionType.Abs
                    )
                nc.scalar.activation(
                    out=p_t,
                    in_=p_t,
                    func=mybir.ActivationFunctionType.Ln,
                    accum_out=vsum_t[:, i : i + 1],
                )
            for s in range(N_SEG):
                nc.vector.tensor_copy(
                    out=cnt_sb[:, s * MM_N : (s + 1) * MM_N], in_=psum_seg[s]
                )
            # Ship the bulk of vsum while the tail tiles still compute;
            # only the last few columns remain for the final tiny DMA.
            head = NTILES - 3
            nc.sync.dma_start(out=vsum_out[:, :head], in_=vsum_t[:, :head])
            nc.sync.dma_start(out=tsum_out[:], in_=cnt_sb)
            nc.sync.dma_start(out=vsum_out[:, head:], in_=vsum_t[:, head:])
    nc.compile()
    return nc


def _get_nc():
    global _NC_CACHE
    if _NC_CACHE is None:
        _NC_CACHE = _build_nc()
    return _NC_CACHE


def _run_device(input, target, **spmd_kwargs):
    p_flat = np.ascontiguousarray(input, dtype=np.float32).reshape(-1)
    t_flat = np.ascontiguousarray(target, dtype=np.float32).reshape(-1)
    in_maps = []
    for k in range(N_CORES):
        sl = slice(k * PER_CORE, (k + 1) * PER_CORE)
        in_maps.append({"p_in": p_flat[sl], "t_in": t_flat[sl]})
    return run_bass_kernel_spmd(nc=_get_nc(), in_maps=in_maps,
                                core_ids=list(range(N_CORES)), **spmd_kwargs)


def _epilogue(results):
    sum_v = np.zeros(C, dtype=np.float64)
    sum_t = np.zeros(C, dtype=np.float64)
    for k in range(N_CORES):
        vs = results[k]["vsum"].astype(np.float64)   # [P, NTILES]
        ts = results[k]["tsum"].astype(np.float64)   # [1, N_SEG*MM_N]
        off = 0
        for i, n in enumerate(TILE_ELEMS):
            g = k * PER_CORE + off
            off += n
            ch = (g // SPATIAL) % C
            scale = 0.5 if i in SQUARE_TILES else 1.0
            sum_v[ch] += scale * vs[:, i].sum()
        for s in range(N_SEG):
            ch = ((k * N_SEG + s) // 2) % C
            sum_t[ch] += ts[0, s * MM_N : (s + 1) * MM_N].sum()
    total = float(M_PER_CH)
    ones = sum_t
    weight = np.where(ones > 0, total / np.maximum(ones, 1.0), EMPTY_WEIGHT)
    bce = -sum_v / total
    return np.asarray((weight * bce).mean(), dtype=np.float32)


def kernel(input, target):
    res = _run_device(input, target)
    return _epilogue(res.results)


# revision 22
# speedup vs baseline: 1.0636x; 1.0636x over previous
"""Trainium2 Bass kernel for CustomBCELoss.

Reference semantics (per torch BCELoss with per-channel weighting):
    p, t flattened channel-first to (C=3, M=8388608)
    ones[c]   = count_nonzero(t[c])
    weight[c] = M / max(ones[c], 1)  if ones[c] > 0 else 1000.0
    bce[c]    = -mean(t*max(log p, -100) + (1-t)*max(log1p(-p), -100))
    out       = mean(weight * bce)

Since t ∈ {0,1}, the per-element term is log|p + t - 1|, and with
p ∈ [1e-4, 1-1e-4] (post-sigmoid probabilities) the -100 clamp never
fires: |p + t - 1| >= ~6e-5 so log >= ~-10.

8-way data-parallel over the flat element range. Per-core pipeline over
[128, 2048] tiles (tapered at both ends) with deep (bufs=6)
double-buffering so the ~430 GB/s/core HBM stream (25.2 MB) never
stalls on slot recycling. Engine balance, each stream < the 60 us DMA:
    Sync DGE : all HBM loads
    PE   : per-segment positive counts. t is exactly 0.0f/1.0f, so the
           strided bf16 view of its high 2 bytes is exactly 0.0/1.0 —
           single-pass bf16 matmuls (ones.T @ t) into PSUM, exact.
    DVE  : d = (p - 1) + t (fused); for SQUARE_TILES also u = d*d
           (their Ln accumulates 2*log|d|, halved on the host)
    ACT  : u = |d| for the remaining tiles; Ln(u) in place with fused
           per-partition accum_out. A dummy Ln in the preamble pins the
           natural_log table set (contains Abs+Ln): one load, preamble.
Tiles never cross an (n, c) half-block boundary, so per-tile/per-segment
partials map 1:1 to channels on the host, which applies the tiny
weight/mean epilogue in float64.
"""

import numpy as np

import concourse.bacc as bacc
import concourse.bass as bass
import concourse.tile as tile
from concourse import mybir
from concourse.bass_utils import run_bass_kernel_spmd

N_CORES = 8
C = 3
SPATIAL = 128 * 128 * 128            # elements per (n, c) block
N_BATCH = 4
FULL = N_BATCH * C * SPATIAL         # 25_165_824 total elements
PER_CORE = FULL // N_CORES           # 3_145_728
P = 128
# Per-partition column counts per tile; sum must equal PER_CORE / P = 24576.
TILE_F = [4096, 4096, 4096, 2048, 2048, 2048, 2048, 2048, 1024, 512, 512]
NTILES = len(TILE_F)
TILE_ELEMS = [P * f for f in TILE_F]
assert sum(TILE_ELEMS) == PER_CORE
# |d| as d*d on DVE for these tiles (rest: Abs on ACT), placed so that
# neither engine carries a backlog into the drain after the last DMA.
SQUARE_TILES = {1, 3, 5, 6, 8}
HALF_BLOCK_COLS = (SPATIAL // 2) // P          # 8192 cols per half-block
N_SEG = (PER_CORE // P) // HALF_BLOCK_COLS     # 3 segments per core
MM_N = 512                                      # matmul moving free dim
M_PER_CH = FULL // C                 # 8_388_608
EMPTY_WEIGHT = 1000.0

_NC_CACHE = None


def _build_nc():
    nc = bacc.Bacc(
        "TRN2", target_bir_lowering=False, debug=False, num_devices=N_CORES
    )
    p_in = nc.declare_dram_parameter(
        "p_in", [PER_CORE], mybir.dt.float32, isOutput=False
    )
    t_in = nc.declare_dram_parameter(
        "t_in", [PER_CORE], mybir.dt.float32, isOutput=False
    )
    vsum_out = nc.declare_dram_parameter(
        "vsum", [P, NTILES], mybir.dt.float32, isOutput=True
    )
    tsum_out = nc.declare_dram_parameter(
        "tsum", [1, N_SEG * MM_N], mybir.dt.float32, isOutput=True
    )

    seg_of_tile = []
    off = 0
    for f in TILE_F:
        assert off // HALF_BLOCK_COLS == (off + f - 1) // HALF_BLOCK_COLS
        seg_of_tile.append(off // HALF_BLOCK_COLS)
        off += f
    mm_total = {s: 0 for s in range(N_SEG)}
    for i, f in enumerate(TILE_F):
        mm_total[seg_of_tile[i]] += f // MM_N

    with tile.TileContext(nc) as tc:
        with (
            tc.tile_pool(name="pp", bufs=6) as p_pool,
            tc.tile_pool(name="tp", bufs=6) as t_pool,
            tc.tile_pool(name="res", bufs=1) as res_pool,
            tc.tile_pool(name="ps", bufs=1, space="PSUM") as ps_pool,
        ):
            ones_t = res_pool.tile([P, 1], mybir.dt.bfloat16)
            nc.vector.memset(ones_t, 1.0)
            vsum_t = res_pool.tile([P, NTILES], mybir.dt.float32)
            cnt_sb = res_pool.tile([1, N_SEG * MM_N], mybir.dt.float32)
            # Dummy Ln pins the natural_log table set (contains Abs too).
            warm_t = res_pool.tile([P, 1], mybir.dt.float32)
            nc.vector.memset(warm_t, 1.0)
            nc.scalar.activation(
                out=warm_t, in_=warm_t, func=mybir.ActivationFunctionType.Ln
            )
            psum_seg = [
                ps_pool.tile(
                    [1, MM_N], mybir.dt.float32, tag=f"seg{s}", name=f"psum_seg{s}"
                )
                for s in range(N_SEG)
            ]
            mm_done = {s: 0 for s in range(N_SEG)}
            off = 0
            for i, f in enumerate(TILE_F):
                n = P * f
                p_src = p_in[off : off + n].rearrange("(p f) -> p f", p=P)
                t_src = t_in[off : off + n].rearrange("(p f) -> p f", p=P)
                off += n
                s = seg_of_tile[i]
                p_t = p_pool.tile([P, f], mybir.dt.float32, tag="p")
                t_t = t_pool.tile([P, f], mybir.dt.float32, tag="t")
                nc.sync.dma_start(out=p_t, in_=p_src)
                nc.sync.dma_start(out=t_t, in_=t_src)
                t_hi = t_t[:].bitcast(mybir.dt.bfloat16).rearrange(
                    "p (f two) -> p f two", two=2
                )[:, :, 1]
                for j in range(f // MM_N):
                    nc.tensor.matmul(
                        out=psum_seg[s][:, :],
                        lhsT=ones_t[:, :],
                        rhs=t_hi[:, j * MM_N : (j + 1) * MM_N],
                        start=(mm_done[s] == 0),
                        stop=(mm_done[s] == mm_total[s] - 1),
                    )
                    mm_done[s] += 1
                # d = (p - 1) + t, in place into p_t
                nc.vector.scalar_tensor_tensor(
                    out=p_t,
                    in0=p_t,
                    scalar=1.0,
                    in1=t_t,
                    op0=mybir.AluOpType.subtract,
                    op1=mybir.AluOpType.add,
                )
                if i in SQUARE_TILES:
                    nc.vector.tensor_tensor(
                        out=p_t, in0=p_t, in1=p_t, op=mybir.AluOpType.mult
                    )
                else:
                    nc.scalar.activation(
                        out=p_t, in_=p_t, func=mybir.ActivationFunctionType.Abs
                    )
                nc.scalar.activation(
                    out=p_t,
                    in_=p_t,
                    func=mybir.ActivationFunctionType.Ln,
                    accum_out=vsum_t[:, i : i + 1],
                )
            for s in range(N_SEG):
                nc.vector.tensor_copy(
                    out=cnt_sb[:, s * MM_N : (s + 1) * MM_N], in_=psum_seg[s]
                )
            nc.sync.dma_start(out=vsum_out[:], in_=vsum_t)
            nc.sync.dma_start(out=tsum_out[:], in_=cnt_sb)
    nc.compile()
    return nc


def _get_nc():
    global _NC_CACHE
    if _NC_CACHE is None:
        _NC_CACHE = _build_nc()
    return _NC_CACHE


def _run_device(input, target, **spmd_kwargs):
    p_flat = np.ascontiguousarray(input, dtype=np.float32).reshape(-1)
    t_flat = np.ascontiguousarray(target, dtype=np.float32).reshape(-1)
    in_maps = []
    for k in range(N_CORES):
        sl = slice(k * PER_CORE, (k + 1) * PER_CORE)
        in_maps.append({"p_in": p_flat[sl], "t_in": t_flat[sl]})
    return run_bass_kernel_spmd(nc=_get_nc(), in_maps=in_maps,
                                core_ids=list(range(N_CORES)), **spmd_kwargs)


def _epilogue(results):
    sum_v = np.zeros(C, dtype=np.float64)
    sum_t = np.zeros(C, dtype=np.float64)
    for k in range(N_CORES):
        vs = results[k]["vsum"].astype(np.float64)   # [P, NTILES]
        ts = results[k]["tsum"].astype(np.float64)   # [1, N_SEG*MM_N]
        off = 0
        for i, n in enumerate(TILE_ELEMS):
            g = k * PER_CORE + off
            off += n
            ch = (g // SPATIAL) % C
            scale = 0.5 if i in SQUARE_TILES else 1.0
            sum_v[ch] += scale * vs[:, i].sum()
        for s in range(N_SEG):
            ch = ((k * N_SEG + s) // 2) % C
            sum_t[ch] += ts[0, s * MM_N : (s + 1) * MM_N].sum()
    total = float(M_PER_CH)
    ones = sum_t
    weight = np.where(ones > 0, total / np.maximum(ones, 1.0), EMPTY_WEIGHT)
    bce = -sum_v / total
    return np.asarray((weight * bce).mean(), dtype=np.float32)


def kernel(input, target):
    res = _run_device(input, target)
    return _epilogue(res.results)


# revision 23
# speedup vs baseline: 1.0754x; 1.0111x over previous
"""Trainium2 Bass kernel for CustomBCELoss.

Reference semantics (per torch BCELoss with per-channel weighting):
    p, t flattened channel-first to (C=3, M=8388608)
    ones[c]   = count_nonzero(t[c])
    weight[c] = M / max(ones[c], 1)  if ones[c] > 0 else 1000.0
    bce[c]    = -mean(t*max(log p, -100) + (1-t)*max(log1p(-p), -100))
    out       = mean(weight * bce)

Since t ∈ {0,1}, the per-element term is log|p + t - 1|, and with
p ∈ [1e-4, 1-1e-4] (post-sigmoid probabilities) the -100 clamp never
fires: |p + t - 1| >= ~6e-5 so log >= ~-10.

8-way data-parallel over the flat element range. Per-core pipeline over
[128, 2048] tiles (tapered at both ends) with deep (bufs=6)
double-buffering so the ~430 GB/s/core HBM stream (25.2 MB) never
stalls on slot recycling. Engine balance, each stream < the 60 us DMA:
    Sync DGE : all HBM loads
    PE   : per-segment positive counts. t is exactly 0.0f/1.0f, so the
           strided bf16 view of its high 2 bytes is exactly 0.0/1.0 —
           single-pass bf16 matmuls (ones.T @ t) into PSUM, exact.
    DVE  : d = (p - 1) + t (fused); for SQUARE_TILES also u = d*d
           (their Ln accumulates 2*log|d|, halved on the host)
    ACT  : u = |d| for the remaining tiles; Ln(u) in place with fused
           per-partition accum_out. A dummy Ln in the preamble pins the
           natural_log table set (contains Abs+Ln): one load, preamble.
Tiles never cross an (n, c) half-block boundary, so per-tile/per-segment
partials map 1:1 to channels on the host, which applies the tiny
weight/mean epilogue in float64.
"""

import numpy as np

import concourse.bacc as bacc
import concourse.bass as bass
import concourse.tile as tile
from concourse import mybir
from concourse.bass_utils import run_bass_kernel_spmd

N_CORES = 8
C = 3
SPATIAL = 128 * 128 * 128            # elements per (n, c) block
N_BATCH = 4
FULL = N_BATCH * C * SPATIAL         # 25_165_824 total elements
PER_CORE = FULL // N_CORES           # 3_145_728
P = 128
# Per-partition column counts per tile; sum must equal PER_CORE / P = 24576.
TILE_F = [4096, 4096, 4096, 2048, 2048, 2048, 2048, 2048, 1024, 512, 512]
NTILES = len(TILE_F)
TILE_ELEMS = [P * f for f in TILE_F]
assert sum(TILE_ELEMS) == PER_CORE
# |d| as d*d on DVE for these tiles (rest: Abs on ACT). ACT keeps Abs
# only on two early big tiles (it has slack while the pipe fills); from
# there on ACT is ln-only so it never carries a backlog into the drain.
SQUARE_TILES = set(range(NTILES)) - {0, 2}
HALF_BLOCK_COLS = (SPATIAL // 2) // P          # 8192 cols per half-block
N_SEG = (PER_CORE // P) // HALF_BLOCK_COLS     # 3 segments per core
MM_N = 512                                      # matmul moving free dim
M_PER_CH = FULL // C                 # 8_388_608
EMPTY_WEIGHT = 1000.0

_NC_CACHE = None


def _build_nc():
    nc = bacc.Bacc(
        "TRN2", target_bir_lowering=False, debug=False, num_devices=N_CORES
    )
    p_in = nc.declare_dram_parameter(
        "p_in", [PER_CORE], mybir.dt.float32, isOutput=False
    )
    t_in = nc.declare_dram_parameter(
        "t_in", [PER_CORE], mybir.dt.float32, isOutput=False
    )
    vsum_out = nc.declare_dram_parameter(
        "vsum", [P, NTILES], mybir.dt.float32, isOutput=True
    )
    tsum_out = nc.declare_dram_parameter(
        "tsum", [1, N_SEG * MM_N], mybir.dt.float32, isOutput=True
    )

    seg_of_tile = []
    off = 0
    for f in TILE_F:
        assert off // HALF_BLOCK_COLS == (off + f - 1) // HALF_BLOCK_COLS
        seg_of_tile.append(off // HALF_BLOCK_COLS)
        off += f
    mm_total = {s: 0 for s in range(N_SEG)}
    for i, f in enumerate(TILE_F):
        mm_total[seg_of_tile[i]] += f // MM_N

    with tile.TileContext(nc) as tc:
        with (
            tc.tile_pool(name="pp", bufs=6) as p_pool,
            tc.tile_pool(name="tp", bufs=6) as t_pool,
            tc.tile_pool(name="res", bufs=1) as res_pool,
            tc.tile_pool(name="ps", bufs=1, space="PSUM") as ps_pool,
        ):
            ones_t = res_pool.tile([P, 1], mybir.dt.bfloat16)
            nc.vector.memset(ones_t, 1.0)
            vsum_t = res_pool.tile([P, NTILES], mybir.dt.float32)
            cnt_sb = res_pool.tile([1, N_SEG * MM_N], mybir.dt.float32)
            # Dummy Ln pins the natural_log table set (contains Abs too).
            warm_t = res_pool.tile([P, 1], mybir.dt.float32)
            nc.vector.memset(warm_t, 1.0)
            nc.scalar.activation(
                out=warm_t, in_=warm_t, func=mybir.ActivationFunctionType.Ln
            )
            psum_seg = [
                ps_pool.tile(
                    [1, MM_N], mybir.dt.float32, tag=f"seg{s}", name=f"psum_seg{s}"
                )
                for s in range(N_SEG)
            ]
            mm_done = {s: 0 for s in range(N_SEG)}
            off = 0
            for i, f in enumerate(TILE_F):
                n = P * f
                p_src = p_in[off : off + n].rearrange("(p f) -> p f", p=P)
                t_src = t_in[off : off + n].rearrange("(p f) -> p f", p=P)
                off += n
                s = seg_of_tile[i]
                p_t = p_pool.tile([P, f], mybir.dt.float32, tag="p")
                t_t = t_pool.tile([P, f], mybir.dt.float32, tag="t")
                nc.sync.dma_start(out=p_t, in_=p_src)
                nc.sync.dma_start(out=t_t, in_=t_src)
                t_hi = t_t[:].bitcast(mybir.dt.bfloat16).rearrange(
                    "p (f two) -> p f two", two=2
                )[:, :, 1]
                for j in range(f // MM_N):
                    nc.tensor.matmul(
                        out=psum_seg[s][:, :],
                        lhsT=ones_t[:, :],
                        rhs=t_hi[:, j * MM_N : (j + 1) * MM_N],
                        start=(mm_done[s] == 0),
                        stop=(mm_done[s] == mm_total[s] - 1),
                    )
                    mm_done[s] += 1
                # d = (p - 1) + t, in place into p_t
                nc.vector.scalar_tensor_tensor(
                    out=p_t,
                    in0=p_t,
                    scalar=1.0,
                    in1=t_t,
                    op0=mybir.AluOpType.subtract,
                    op1=mybir.AluOpType.add,
                )
                if i in SQUARE_TILES:
                    nc.vector.tensor_tensor(
                        out=p_t, in0=p_t, in1=p_t, op=mybir.AluOpType.mult
                    )
                else:
                    nc.scalar.activation(
                        out=p_t, in_=p_t, func=mybir.ActivationFunctionType.Abs
                    )
                nc.scalar.activation(
                    out=p_t,
                    in_=p_t,
                    func=mybir.ActivationFunctionType.Ln,
                    accum_out=vsum_t[:, i : i + 1],
                )
            for s in range(N_SEG):
                nc.vector.tensor_copy(
                    out=cnt_sb[:, s * MM_N : (s + 1) * MM_N], in_=psum_seg[s]
                )
            nc.sync.dma_start(out=vsum_out[:], in_=vsum_t)
            nc.sync.dma_start(out=tsum_out[:], in_=cnt_sb)
    nc.compile()
    return nc


def _get_nc():
    global _NC_CACHE
    if _NC_CACHE is None:
        _NC_CACHE = _build_nc()
    return _NC_CACHE


def _run_device(input, target, **spmd_kwargs):
    p_flat = np.ascontiguousarray(input, dtype=np.float32).reshape(-1)
    t_flat = np.ascontiguousarray(target, dtype=np.float32).reshape(-1)
    in_maps = []
    for k in range(N_CORES):
        sl = slice(k * PER_CORE, (k + 1) * PER_CORE)
        in_maps.append({"p_in": p_flat[sl], "t_in": t_flat[sl]})
    return run_bass_kernel_spmd(nc=_get_nc(), in_maps=in_maps,
                                core_ids=list(range(N_CORES)), **spmd_kwargs)


def _epilogue(results):
    sum_v = np.zeros(C, dtype=np.float64)
    sum_t = np.zeros(C, dtype=np.float64)
    for k in range(N_CORES):
        vs = results[k]["vsum"].astype(np.float64)   # [P, NTILES]
        ts = results[k]["tsum"].astype(np.float64)   # [1, N_SEG*MM_N]
        off = 0
        for i, n in enumerate(TILE_ELEMS):
            g = k * PER_CORE + off
            off += n
            ch = (g // SPATIAL) % C
            scale = 0.5 if i in SQUARE_TILES else 1.0
            sum_v[ch] += scale * vs[:, i].sum()
        for s in range(N_SEG):
            ch = ((k * N_SEG + s) // 2) % C
            sum_t[ch] += ts[0, s * MM_N : (s + 1) * MM_N].sum()
    total = float(M_PER_CH)
    ones = sum_t
    weight = np.where(ones > 0, total / np.maximum(ones, 1.0), EMPTY_WEIGHT)
    bce = -sum_v / total
    return np.asarray((weight * bce).mean(), dtype=np.float32)


def kernel(input, target):
    res = _run_device(input, target)
    return _epilogue(res.results)
